# revision 1
# baseline (speedup 1.0000x reference)
import numpy as np

# nn_GaussianMixture: log-likelihood of N points under an M-component GMM.
# Shapes hardcoded per contract: points [500000,16], centers [128,16],
# covs_inv_sqrt [128,16,16], weights [128], threshold [1].
N, M, D = 500000, 128, 16
N_CORES = 8


def _prep(centers, covs_inv_sqrt, weights):
    centers = np.asarray(centers, np.float32)
    L = np.asarray(covs_inv_sqrt, np.float32)
    w = np.abs(np.asarray(weights, np.float32))
    covs_inv = np.einsum('jde,jfe->jdf', L, L).astype(np.float32)      # [M,D,D]
    center_prs = w / (w.sum() + 1e-30)
    sign, logdet = np.linalg.slogdet(covs_inv.astype(np.float64))
    logcoefs = (np.log(center_prs.astype(np.float64) + 1e-300)
                + 0.5 * logdet).astype(np.float32)                      # [M]
    S_flat = covs_inv.reshape(M, D * D)                                 # [M,256]
    Sc = np.einsum('jde,je->jd', covs_inv, centers).astype(np.float32)  # [M,D]
    cSc = np.einsum('jd,jd->j', centers, Sc).astype(np.float32)         # [M]
    return S_flat, Sc, cSc, logcoefs


def _shard_numpy(p, S_flat, Sc, cSc, logcoefs):
    n = p.shape[0]
    out = np.empty((n, 1), np.float32)
    # q_ij = xx.S_flat - 2 x.Sc + cSc ; d_ij = -0.5 q + logcoef ; logsumexp_j
    for s in range(0, n, 8192):
        pe = p[s:s + 8192]
        xx = (pe[:, :, None] * pe[:, None, :]).reshape(pe.shape[0], D * D)
        q = xx @ S_flat.T
        q -= 2.0 * (pe @ Sc.T)
        q += cSc[None, :]
        d = -0.5 * q + logcoefs[None, :]
        mx = d.max(axis=1, keepdims=True)
        out[s:s + 8192] = mx + np.log(np.exp(d - mx).sum(axis=1, keepdims=True))
    return out


def _try_jax(points, S_flat, Sc, cSc, logcoefs):
    import jax
    import jax.numpy as jnp
    devs = jax.devices()
    if len(devs) < N_CORES:
        raise RuntimeError("need 8 cores")

    def f(p, Sf, Sc_, cSc_, lc):
        xx = (p[:, :, None] * p[:, None, :]).reshape(p.shape[0], D * D)
        q = xx @ Sf.T - 2.0 * (p @ Sc_.T) + cSc_[None, :]
        d = -0.5 * q + lc[None, :]
        mx = jnp.max(d, axis=1, keepdims=True)
        return mx + jnp.log(jnp.sum(jnp.exp(d - mx), axis=1, keepdims=True))

    shards = np.split(points, N_CORES)
    jf = jax.jit(f)
    res = []
    for s, dv in zip(shards, devs[:N_CORES]):
        a = [jax.device_put(x, dv) for x in (s, S_flat, Sc, cSc, logcoefs)]
        res.append(jf(*a))
    return np.concatenate([np.asarray(r) for r in res], axis=0)


def kernel(points, centers, covs_inv_sqrt, weights, threshold):
    points = np.asarray(points, np.float32)
    S_flat, Sc, cSc, logcoefs = _prep(centers, covs_inv_sqrt, weights)
    try:
        out = _try_jax(points, S_flat, Sc, cSc, logcoefs)
    except Exception:
        out = _shard_numpy(points, S_flat, Sc, cSc, logcoefs)
    return (out - np.asarray(threshold, np.float32)[None, :]).astype(np.float32)

